# revision 32
# baseline (speedup 1.0000x reference)
"""Trainium2 Bass kernel for: Conv3d(3->16, k=3, VALID) -> min over depth -> softmax(channels).

Full inputs:  x [8, 3, 24, 128, 128] f32, conv_weight [16, 3, 3, 3, 3] f32
Full output:  [8, 16, 126, 126] f32
Sharding: data-parallel over batch, one sample per NeuronCore (8 cores).

Per-core scheme (H-packed im2col, fp16 matmuls, K padded to 128):
  - 16 h-blocks b; block handles h_out in [8b, 8b+8) (last block: 6 rows).
  - Host pre-packs x into x5 [16, 128, 24, 126] fp16:
    x5[b, (c*3+kw)*hh_n + hh, d, w] = x[c, d, 8b+hh, kw+w], rows 90.. zero.
    (3.75x replication; kd taps need no replication - depth lives in the free
    axis and is handled by shifted rhs offsets with PSUM accumulation.)
    One fully-contiguous 128-partition DMA per block => ~200 GB/s.
  - lhsT_p [128, M=(h_l,oc)] = W[oc, c, p, hh-h_l, kw] (0 <= hh-h_l < 3), p=kd.
    K=128 (zero-padded) keeps the PE xbus at full rate + enables FWL.
  - Depth quads DQ (5x4 + 1x2, no overlap waste); per quad one PSUM bank
    [M, nd, 126] accumulates the 3 kd passes (start/stop flags).
  - min over depth: per-quad DVE reduce_min + pairwise tree -> per-group mn.
  - softmax over the 16 channels per partition group of 16, as
    exp(mn - ln(sum exp)): exp/Ln on the otherwise-idle ScalarE, the
    channel-sum-and-broadcast as ONE fp16 PE matmul with a 0/1 block matrix
    (ob[k,p] = 1 iff k//16 == p//16), and only a subtract on VectorE.
    The whole chain is software-pipelined across the conv stream in groups
    of 4 h-blocks, each step emitted a few blocks after its input
    is ready so the in-order engine queues never stall; output DMAs ride
    the scalar queue so they never wait behind the big input loads.
"""

import functools
import os
import sys

import numpy as np

os.environ.setdefault("MYCRO_LOCAL_CACHE", "1")
if os.path.isdir("/opt/trn_rl_repo") and "/opt/trn_rl_repo" not in sys.path:
    sys.path.insert(0, "/opt/trn_rl_repo")

import concourse.bacc as bacc
import concourse.mybir as mybir
import concourse.tile as tile
from concourse import bass_utils

C, D, H, W = 3, 24, 128, 128
OC, KD, KH, KW = 16, 3, 3, 3
DO, HO, WO = D - 2, H - 2, W - 2  # 22, 126, 126
DQ = ((0, 4), (4, 4), (8, 4), (12, 4), (16, 4), (20, 2))  # (start, ndepth)
NCORES = 8
NBLK = 16  # h blocks: 15 full (8 rows) + 1 tail (6 rows)
GROUPS = ((0, 4), (4, 4), (8, 4), (12, 4))  # softmax groups (start, size)
F32 = mybir.dt.float32
F16 = mybir.dt.float16


def _pack_weights(w: np.ndarray):
    """lhsT/lhsT_last [KD,128,128] (zero-padded K and M) + ob [128,128]."""

    def pack(nh):
        hh_n = nh + 2
        lhsT = np.zeros((KD, 128, 128), dtype=np.float32)
        for p in range(KD):
            for c in range(C):
                for kw in range(KW):
                    for hh in range(hh_n):
                        r = (c * KW + kw) * hh_n + hh
                        for hl in range(nh):
                            kh = hh - hl
                            if 0 <= kh < KH:
                                lhsT[p, r, hl * OC : hl * OC + OC] = w[:, c, p, kh, kw]
        return lhsT

    ob = np.zeros((128, 128), dtype=np.float32)
    for pp in range(128):
        g0 = (pp // OC) * OC
        ob[pp, g0 : g0 + OC] = 1.0
    return pack(8), pack(6), ob


def _pack_x5(x1: np.ndarray) -> np.ndarray:
    """x [3,24,128,128] f32 -> x5 [NBLK,128,24,126] f16 (padded rows zero)."""
    x5 = np.zeros((NBLK, 128, D, WO), dtype=np.float16)
    for b in range(NBLK):
        nh = 8 if b < NBLK - 1 else 6
        hh_n = nh + 2
        for c in range(C):
            for kw in range(KW):
                r0 = (c * KW + kw) * hh_n
                # [hh, d, w] <- x[c, d, 8b+hh, kw+w]
                x5[b, r0 : r0 + hh_n] = np.transpose(
                    x1[c, :, 8 * b : 8 * b + hh_n, kw : kw + WO], (1, 0, 2)
                )
    return x5


def build_program(reps: int = 1, stage2: str = "full"):
    """reps > 1 wraps the per-sample body in a hardware loop (dev timing only).
    stage2: none | exp | smmm | full (dev bisection of the softmax tail)."""
    nc = bacc.Bacc(
        "TRN2",
        target_bir_lowering=False,
        debug=False,
        enable_asserts=True,
        num_devices=NCORES,
    )
    x5_d = nc.dram_tensor("x5", [NBLK, 128, D, WO], F16, kind="ExternalInput").ap()
    lw_d = nc.dram_tensor("lw", [KD, 128, 128], F16, kind="ExternalInput").ap()
    lwl_d = nc.dram_tensor("lwl", [KD, 128, 128], F16, kind="ExternalInput").ap()
    ob_d = nc.dram_tensor("ob", [128, 128], F16, kind="ExternalInput").ap()
    y_d = nc.dram_tensor("y", [OC, HO, WO], F32, kind="ExternalOutput").ap()

    with tile.TileContext(nc) as tc:
        with (
            tc.tile_pool(name="const", bufs=1) as cpool,
            tc.tile_pool(name="xt", bufs=4) as xpool,
            tc.tile_pool(name="sm", bufs=3) as spool,
            tc.tile_pool(name="qps", bufs=6, space="PSUM") as qpool,
            tc.tile_pool(name="sps", bufs=2, space="PSUM") as smpool,
        ):
            lw_sb = cpool.tile([128, KD, 128], F16)
            nc.sync.dma_start(lw_sb[:], lw_d.rearrange("p r m -> r p m").bitcast(F16))
            lwl_sb = cpool.tile([128, KD, 128], F16)
            nc.sync.dma_start(lwl_sb[:], lwl_d.rearrange("p r m -> r p m").bitcast(F16))
            ob_sb = cpool.tile([128, 128], F16)
            nc.sync.dma_start(ob_sb[:], ob_d)

            def emit_body():
                state = {}  # per softmax group g: mn/et/st/ot tiles

                # Softmax over 4-block groups, software-pipelined across the
                # conv stream: each op is emitted a few blocks after its input
                # became available, so the in-order PE/DVE/ACT queues never
                # stall on cross-engine latency.
                def softmax_step(step, g):
                    g0, gsz = GROUPS[g]
                    if step == 0 and stage2 != "none":
                        et = spool.tile([128, gsz, WO], F16, tag="et", bufs=3, name=f"et{g}")
                        nc.scalar.activation(
                            et[:], state[g]["mn"][:], mybir.ActivationFunctionType.Exp
                        )
                        state[g]["et"] = et
                    if stage2 in ("none", "exp"):
                        return
                    eg = state[g]["et"][:]
                    if step == 1:
                        # group-sum broadcast to all 128 partitions in one MM:
                        # ob[k, p] = 1 iff k//16 == p//16
                        st = smpool.tile([128, gsz, WO], F32, tag="ss", name=f"st{g}")
                        nc.tensor.matmul(st[:], ob_sb[:], eg, start=True, stop=True)
                        state[g]["st"] = st
                    elif step == 2:
                        # softmax = exp(mn - ln(sum)); Ln/Exp ride the idle
                        # ScalarE instead of reciprocal+mult on the busy DVE
                        lt = spool.tile([128, gsz, WO], F32, tag="lt", bufs=2, name=f"lt{g}")
                        nc.scalar.activation(
                            lt[:], state[g]["st"][:], mybir.ActivationFunctionType.Ln
                        )
                        state[g]["lt"] = lt
                    elif step == 3:
                        dt = spool.tile([128, gsz, WO], F32, tag="dt", bufs=2, name=f"dt{g}")
                        nc.vector.tensor_tensor(
                            dt[:], state[g]["mn"][:], state[g]["lt"][:],
                            op=mybir.AluOpType.subtract,
                        )
                        state[g]["dt"] = dt
                    elif step == 4:
                        if stage2 == "smmm":
                            return
                        ot = spool.tile([128, gsz, WO], F32, tag="ot", bufs=2, name=f"ot{g}")
                        nc.scalar.activation(
                            ot[:], state[g]["dt"][:], mybir.ActivationFunctionType.Exp
                        )
                        for j in range(gsz):
                            bb_ = g0 + j
                            nh = 8 if bb_ < NBLK - 1 else 6
                            dst = y_d[:, 8 * bb_ : 8 * bb_ + nh, :].rearrange(
                                "oc h w -> h oc w"
                            )
                            nc.scalar.dma_start(dst, ot[: nh * OC, j, :])

                # schedule[B] = list of (step, g) to emit before conv block B
                # (exp right when its blocks' mins exist; the rest spaced a
                # few blocks later so the in-order engine queues never stall)
                schedule = {}
                for g, (g0, gsz) in enumerate(GROUPS):
                    end = g0 + gsz - 1
                    for step, off in enumerate((1, 4, 5, 6, 7)):
                        schedule.setdefault(end + off, []).append((step, g))

                blk2grp = {}
                for g, (g0, gsz) in enumerate(GROUPS):
                    for b in range(g0, g0 + gsz):
                        blk2grp[b] = g

                for b in range(NBLK):
                    m_n = 128
                    g_cur = blk2grp[b]
                    g0, gsz = GROUPS[g_cur]
                    if b == g0:
                        state[g_cur] = {
                            "mn": spool.tile([128, gsz, WO], F32, tag="mn", bufs=3, name=f"mn{g_cur}")
                        }
                    lw_t = lw_sb if b < NBLK - 1 else lwl_sb
                    for step, g in schedule.get(b, []):
                        softmax_step(step, g)

                    xt = xpool.tile([128, D, WO], F16, tag="xt")
                    nc.sync.dma_start(xt[:], x5_d[b].bitcast(F16))

                    mins = []
                    for q, (dq, nd) in enumerate(DQ):
                        pt = qpool.tile([m_n, nd, WO], F32, tag="q")
                        for p in range(KD):
                            nc.tensor.matmul(
                                pt[:],
                                lw_t[:, p, :m_n],
                                xt[:, dq + p : dq + p + nd, :],
                                start=(p == 0),
                                stop=(p == KD - 1),
                            )
                        qm = spool.tile([m_n, WO], F32, tag="qm", bufs=14)
                        nc.vector.tensor_reduce(
                            qm[:],
                            pt[:].rearrange("m j w -> m w j"),
                            axis=mybir.AxisListType.X,
                            op=mybir.AluOpType.min,
                        )
                        mins.append(qm)

                    t01 = spool.tile([m_n, WO], F32, tag="tm", bufs=10)
                    nc.vector.tensor_tensor(t01[:], mins[0][:], mins[1][:], op=mybir.AluOpType.min)
                    t23 = spool.tile([m_n, WO], F32, tag="tm", bufs=10)
                    nc.vector.tensor_tensor(t23[:], mins[2][:], mins[3][:], op=mybir.AluOpType.min)
                    t45 = spool.tile([m_n, WO], F32, tag="tm", bufs=10)
                    nc.vector.tensor_tensor(t45[:], mins[4][:], mins[5][:], op=mybir.AluOpType.min)
                    t03 = spool.tile([m_n, WO], F32, tag="tm", bufs=10)
                    nc.vector.tensor_tensor(t03[:], t01[:], t23[:], op=mybir.AluOpType.min)
                    nc.vector.tensor_tensor(
                        state[g_cur]["mn"][:m_n, b - g0, :], t03[:], t45[:],
                        op=mybir.AluOpType.min,
                    )

                # flush softmax steps scheduled past the last conv block
                # (block 15's weights are zero-padded to M=128, so its min
                # slice partitions 96..127 are exact zeros - finite for exp)
                for at in sorted(k for k in schedule if k >= NBLK):
                    for step, g in schedule[at]:
                        softmax_step(step, g)

            if reps == 1:
                emit_body()
            else:
                with tc.For_i(0, reps, 1, hint_engines=(mybir.EngineType.PE,)):
                    emit_body()

    nc.compile()
    return nc


@functools.lru_cache(maxsize=1)
def _program():
    return build_program()


def make_in_maps(x: np.ndarray, w: np.ndarray):
    lw, lwl, ob = _pack_weights(w)
    lw = lw.astype(np.float16)
    lwl = lwl.astype(np.float16)
    return [
        {"x5": _pack_x5(x[i]), "lw": lw, "lwl": lwl, "ob": ob.astype(np.float16)}
        for i in range(x.shape[0])
    ]


def kernel(x, conv_weight):
    x = np.ascontiguousarray(np.asarray(x, dtype=np.float32))
    w = np.ascontiguousarray(np.asarray(conv_weight, dtype=np.float32))
    assert x.shape == (NCORES, C, D, H, W), x.shape
    nc = _program()
    in_maps = make_in_maps(x, w)
    res = bass_utils.run_bass_kernel_spmd(nc, in_maps, core_ids=list(range(NCORES)))
    out = np.stack([res.results[i]["y"] for i in range(NCORES)])
    return out.astype(np.float32)


# revision 34
# speedup vs baseline: 1.1992x; 1.1992x over previous
"""Trainium2 Bass kernel for: Conv3d(3->16, k=3, VALID) -> min over depth -> softmax(channels).

Full inputs:  x [8, 3, 24, 128, 128] f32, conv_weight [16, 3, 3, 3, 3] f32
Full output:  [8, 16, 126, 126] f32
Sharding: data-parallel over batch, one sample per NeuronCore (8 cores).

Per-core scheme (H-packed im2col, fp16 matmuls, K padded to 128):
  - 16 h-blocks b; block handles h_out in [8b, 8b+8) (last block: 6 rows).
  - Host pre-packs x into x5 [16, 128, 24, 126] fp16:
    x5[b, (c*3+kw)*hh_n + hh, d, w] = x[c, d, 8b+hh, kw+w], rows 90.. zero.
    (3.75x replication; kd taps need no replication - depth lives in the free
    axis and is handled by shifted rhs offsets with PSUM accumulation.)
    One fully-contiguous 128-partition DMA per block => ~200 GB/s.
  - lhsT_p [128, M=(h_l,oc)] = W[oc, c, p, hh-h_l, kw] (0 <= hh-h_l < 3), p=kd.
    K=128 (zero-padded) keeps the PE xbus at full rate + enables FWL.
  - Depth quads DQ (5x4 + 1x2, no overlap waste); per quad one PSUM bank
    [M, nd, 126] accumulates the 3 kd passes (start/stop flags).
  - min over depth: per-quad DVE reduce_min + pairwise tree -> per-group mn.
  - softmax over the 16 channels per partition group of 16, as
    exp(mn - ln(sum exp)): exp/Ln on the otherwise-idle ScalarE, the
    channel-sum-and-broadcast as ONE fp16 PE matmul with a 0/1 block matrix
    (ob[k,p] = 1 iff k//16 == p//16), and only a subtract on VectorE.
    The whole chain is software-pipelined across the conv stream in groups
    of (4,4,4,3,1) h-blocks, each step emitted a few blocks after its input
    is ready so the in-order engine queues never stall; output DMAs ride
    the scalar queue so they never wait behind the big input loads.
"""

import functools
import os
import sys

import numpy as np

os.environ.setdefault("MYCRO_LOCAL_CACHE", "1")
if os.path.isdir("/opt/trn_rl_repo") and "/opt/trn_rl_repo" not in sys.path:
    sys.path.insert(0, "/opt/trn_rl_repo")

import concourse.bacc as bacc
import concourse.mybir as mybir
import concourse.tile as tile
from concourse import bass_utils

C, D, H, W = 3, 24, 128, 128
OC, KD, KH, KW = 16, 3, 3, 3
DO, HO, WO = D - 2, H - 2, W - 2  # 22, 126, 126
DQ = ((0, 4), (4, 4), (8, 4), (12, 4), (16, 4), (20, 2))  # (start, ndepth)
NCORES = 8
NBLK = 16  # h blocks: 15 full (8 rows) + 1 tail (6 rows)
GROUPS = ((0, 4), (4, 4), (8, 4), (12, 3), (15, 1))  # softmax groups (start, size)
F32 = mybir.dt.float32
F16 = mybir.dt.float16


def _pack_weights(w: np.ndarray):
    """lhsT/lhsT_last [KD,128,128] (zero-padded K and M) + ob [128,128]."""

    def pack(nh):
        hh_n = nh + 2
        lhsT = np.zeros((KD, 128, 128), dtype=np.float32)
        for p in range(KD):
            for c in range(C):
                for kw in range(KW):
                    for hh in range(hh_n):
                        r = (c * KW + kw) * hh_n + hh
                        for hl in range(nh):
                            kh = hh - hl
                            if 0 <= kh < KH:
                                lhsT[p, r, hl * OC : hl * OC + OC] = w[:, c, p, kh, kw]
        return lhsT

    ob = np.zeros((128, 128), dtype=np.float32)
    for pp in range(128):
        g0 = (pp // OC) * OC
        ob[pp, g0 : g0 + OC] = 1.0
    return pack(8), pack(6), ob


def _pack_x5(x1: np.ndarray) -> np.ndarray:
    """x [3,24,128,128] f32 -> x5 [NBLK,128,24,126] f16 (padded rows zero)."""
    x5 = np.zeros((NBLK, 128, D, WO), dtype=np.float16)
    for b in range(NBLK):
        nh = 8 if b < NBLK - 1 else 6
        hh_n = nh + 2
        for c in range(C):
            for kw in range(KW):
                r0 = (c * KW + kw) * hh_n
                # [hh, d, w] <- x[c, d, 8b+hh, kw+w]
                x5[b, r0 : r0 + hh_n] = np.transpose(
                    x1[c, :, 8 * b : 8 * b + hh_n, kw : kw + WO], (1, 0, 2)
                )
    return x5


def build_program(reps: int = 1, stage2: str = "full"):
    """reps > 1 wraps the per-sample body in a hardware loop (dev timing only).
    stage2: none | exp | smmm | full (dev bisection of the softmax tail)."""
    nc = bacc.Bacc(
        "TRN2",
        target_bir_lowering=False,
        debug=False,
        enable_asserts=True,
        num_devices=NCORES,
    )
    x5_d = nc.dram_tensor("x5", [NBLK, 128, D, WO], F16, kind="ExternalInput").ap()
    lw_d = nc.dram_tensor("lw", [KD, 128, 128], F16, kind="ExternalInput").ap()
    lwl_d = nc.dram_tensor("lwl", [KD, 128, 128], F16, kind="ExternalInput").ap()
    ob_d = nc.dram_tensor("ob", [128, 128], F16, kind="ExternalInput").ap()
    y_d = nc.dram_tensor("y", [HO, OC, WO], F32, kind="ExternalOutput").ap()

    with tile.TileContext(nc) as tc:
        with (
            tc.tile_pool(name="const", bufs=1) as cpool,
            tc.tile_pool(name="xt", bufs=4) as xpool,
            tc.tile_pool(name="sm", bufs=3) as spool,
            tc.tile_pool(name="qps", bufs=6, space="PSUM") as qpool,
            tc.tile_pool(name="sps", bufs=2, space="PSUM") as smpool,
        ):
            lw_sb = cpool.tile([128, KD, 128], F16)
            nc.sync.dma_start(lw_sb[:], lw_d.rearrange("p r m -> r p m").bitcast(F16))
            lwl_sb = cpool.tile([128, KD, 128], F16)
            nc.sync.dma_start(lwl_sb[:], lwl_d.rearrange("p r m -> r p m").bitcast(F16))
            ob_sb = cpool.tile([128, 128], F16)
            nc.sync.dma_start(ob_sb[:], ob_d)

            def emit_body():
                state = {}  # per softmax group g: mn/et/st/ot tiles

                # Softmax over 4-block groups, software-pipelined across the
                # conv stream: each op is emitted a few blocks after its input
                # became available, so the in-order PE/DVE/ACT queues never
                # stall on cross-engine latency.
                def softmax_step(step, g):
                    g0, gsz = GROUPS[g]
                    if step == 0 and stage2 != "none":
                        et = spool.tile([128, gsz, WO], F16, tag="et", bufs=3, name=f"et{g}")
                        nc.scalar.activation(
                            et[:], state[g]["mn"][:], mybir.ActivationFunctionType.Exp
                        )
                        state[g]["et"] = et
                    if stage2 in ("none", "exp"):
                        return
                    eg = state[g]["et"][:]
                    if step == 1:
                        # group-sum broadcast to all 128 partitions in one MM:
                        # ob[k, p] = 1 iff k//16 == p//16
                        st = smpool.tile([128, gsz, WO], F32, tag="ss", name=f"st{g}")
                        nc.tensor.matmul(st[:], ob_sb[:], eg, start=True, stop=True)
                        state[g]["st"] = st
                    elif step == 2:
                        # softmax = exp(mn - ln(sum)); Ln/Exp ride the idle
                        # ScalarE instead of reciprocal+mult on the busy DVE
                        lt = spool.tile([128, gsz, WO], F32, tag="lt", bufs=2, name=f"lt{g}")
                        nc.scalar.activation(
                            lt[:], state[g]["st"][:], mybir.ActivationFunctionType.Ln
                        )
                        state[g]["lt"] = lt
                    elif step == 3:
                        dt = spool.tile([128, gsz, WO], F32, tag="dt", bufs=2, name=f"dt{g}")
                        nc.vector.tensor_tensor(
                            dt[:], state[g]["mn"][:], state[g]["lt"][:],
                            op=mybir.AluOpType.subtract,
                        )
                        state[g]["dt"] = dt
                    elif step == 4:
                        if stage2 == "smmm":
                            return
                        ot = spool.tile([128, gsz, WO], F32, tag="ot", bufs=2, name=f"ot{g}")
                        nc.scalar.activation(
                            ot[:], state[g]["dt"][:], mybir.ActivationFunctionType.Exp
                        )
                        for j in range(gsz):
                            bb_ = g0 + j
                            nh = 8 if bb_ < NBLK - 1 else 6
                            # [h, oc, w] layout: dest is one contiguous burst
                            dst = y_d[8 * bb_ : 8 * bb_ + nh, :, :]
                            nc.scalar.dma_start(dst, ot[: nh * OC, j, :])

                # schedule[B] = list of (step, g) to emit before conv block B
                # (exp right when its blocks' mins exist; the rest spaced a
                # few blocks later so the in-order engine queues never stall)
                schedule = {}
                for g, (g0, gsz) in enumerate(GROUPS):
                    end = g0 + gsz - 1
                    for step, off in enumerate((1, 4, 5, 6, 7)):
                        schedule.setdefault(end + off, []).append((step, g))

                blk2grp = {}
                for g, (g0, gsz) in enumerate(GROUPS):
                    for b in range(g0, g0 + gsz):
                        blk2grp[b] = g

                for b in range(NBLK):
                    m_n = 128
                    g_cur = blk2grp[b]
                    g0, gsz = GROUPS[g_cur]
                    if b == g0:
                        state[g_cur] = {
                            "mn": spool.tile([128, gsz, WO], F32, tag="mn", bufs=3, name=f"mn{g_cur}")
                        }
                    lw_t = lw_sb if b < NBLK - 1 else lwl_sb
                    for step, g in schedule.get(b, []):
                        softmax_step(step, g)

                    xt = xpool.tile([128, D, WO], F16, tag="xt")
                    nc.sync.dma_start(xt[:], x5_d[b].bitcast(F16))

                    mins = []
                    for q, (dq, nd) in enumerate(DQ):
                        pt = qpool.tile([m_n, nd, WO], F32, tag="q")
                        for p in range(KD):
                            nc.tensor.matmul(
                                pt[:],
                                lw_t[:, p, :m_n],
                                xt[:, dq + p : dq + p + nd, :],
                                start=(p == 0),
                                stop=(p == KD - 1),
                            )
                        qm = spool.tile([m_n, WO], F32, tag="qm", bufs=14)
                        nc.vector.tensor_reduce(
                            qm[:],
                            pt[:].rearrange("m j w -> m w j"),
                            axis=mybir.AxisListType.X,
                            op=mybir.AluOpType.min,
                        )
                        mins.append(qm)

                    t01 = spool.tile([m_n, WO], F32, tag="tm", bufs=10)
                    nc.vector.tensor_tensor(t01[:], mins[0][:], mins[1][:], op=mybir.AluOpType.min)
                    t23 = spool.tile([m_n, WO], F32, tag="tm", bufs=10)
                    nc.vector.tensor_tensor(t23[:], mins[2][:], mins[3][:], op=mybir.AluOpType.min)
                    t45 = spool.tile([m_n, WO], F32, tag="tm", bufs=10)
                    nc.vector.tensor_tensor(t45[:], mins[4][:], mins[5][:], op=mybir.AluOpType.min)
                    t03 = spool.tile([m_n, WO], F32, tag="tm", bufs=10)
                    nc.vector.tensor_tensor(t03[:], t01[:], t23[:], op=mybir.AluOpType.min)
                    nc.vector.tensor_tensor(
                        state[g_cur]["mn"][:m_n, b - g0, :], t03[:], t45[:],
                        op=mybir.AluOpType.min,
                    )

                # flush softmax steps scheduled past the last conv block
                # (block 15's weights are zero-padded to M=128, so its min
                # slice partitions 96..127 are exact zeros - finite for exp)
                for at in sorted(k for k in schedule if k >= NBLK):
                    for step, g in schedule[at]:
                        softmax_step(step, g)

            if reps == 1:
                emit_body()
            else:
                with tc.For_i(0, reps, 1, hint_engines=(mybir.EngineType.PE,)):
                    emit_body()

    nc.compile()
    return nc


@functools.lru_cache(maxsize=1)
def _program():
    return build_program()


def make_in_maps(x: np.ndarray, w: np.ndarray):
    lw, lwl, ob = _pack_weights(w)
    lw = lw.astype(np.float16)
    lwl = lwl.astype(np.float16)
    return [
        {"x5": _pack_x5(x[i]), "lw": lw, "lwl": lwl, "ob": ob.astype(np.float16)}
        for i in range(x.shape[0])
    ]


def kernel(x, conv_weight):
    x = np.ascontiguousarray(np.asarray(x, dtype=np.float32))
    w = np.ascontiguousarray(np.asarray(conv_weight, dtype=np.float32))
    assert x.shape == (NCORES, C, D, H, W), x.shape
    nc = _program()
    in_maps = make_in_maps(x, w)
    res = bass_utils.run_bass_kernel_spmd(nc, in_maps, core_ids=list(range(NCORES)))
    # device writes [h, oc, w]; transpose back on host
    out = np.stack(
        [np.transpose(res.results[i]["y"], (1, 0, 2)) for i in range(NCORES)]
    )
    return np.ascontiguousarray(out, dtype=np.float32)


# revision 35
# speedup vs baseline: 1.2005x; 1.0011x over previous
"""Trainium2 Bass kernel for: Conv3d(3->16, k=3, VALID) -> min over depth -> softmax(channels).

Full inputs:  x [8, 3, 24, 128, 128] f32, conv_weight [16, 3, 3, 3, 3] f32
Full output:  [8, 16, 126, 126] f32
Sharding: data-parallel over batch, one sample per NeuronCore (8 cores).

Per-core scheme (H-packed im2col, fp16 matmuls, K padded to 128):
  - 16 h-blocks b; block handles h_out in [8b, 8b+8) (last block: 6 rows).
  - Host pre-packs x into x5 [16, 128, 24, 126] fp16:
    x5[b, (c*3+kw)*hh_n + hh, d, w] = x[c, d, 8b+hh, kw+w], rows 90.. zero.
    (3.75x replication; kd taps need no replication - depth lives in the free
    axis and is handled by shifted rhs offsets with PSUM accumulation.)
    One fully-contiguous 128-partition DMA per block => ~200 GB/s.
  - lhsT_p [128, M=(h_l,oc)] = W[oc, c, p, hh-h_l, kw] (0 <= hh-h_l < 3), p=kd.
    K=128 (zero-padded) keeps the PE xbus at full rate + enables FWL.
  - Depth quads DQ (5x4 + 1x2, no overlap waste); per quad one PSUM bank
    [M, nd, 126] accumulates the 3 kd passes (start/stop flags).
  - min over depth: per-quad DVE reduce_min + pairwise tree -> per-group mn.
  - softmax over the 16 channels per partition group of 16, as
    exp(mn - ln(sum exp)): exp/Ln on the otherwise-idle ScalarE, the
    channel-sum-and-broadcast as ONE fp16 PE matmul with a 0/1 block matrix
    (ob[k,p] = 1 iff k//16 == p//16), and only a subtract on VectorE.
    The whole chain is software-pipelined across the conv stream in groups
    of (4,4,4,3,1) h-blocks, each step emitted a few blocks after its input
    is ready so the in-order engine queues never stall; output DMAs ride
    the scalar queue so they never wait behind the big input loads.
"""

import functools
import os
import sys

import numpy as np

os.environ.setdefault("MYCRO_LOCAL_CACHE", "1")
if os.path.isdir("/opt/trn_rl_repo") and "/opt/trn_rl_repo" not in sys.path:
    sys.path.insert(0, "/opt/trn_rl_repo")

import concourse.bacc as bacc
import concourse.mybir as mybir
import concourse.tile as tile
from concourse import bass_utils

C, D, H, W = 3, 24, 128, 128
OC, KD, KH, KW = 16, 3, 3, 3
DO, HO, WO = D - 2, H - 2, W - 2  # 22, 126, 126
DQ = ((0, 4), (4, 4), (8, 4), (12, 4), (16, 4), (20, 2))  # (start, ndepth)
NCORES = 8
NBLK = 16  # h blocks: 15 full (8 rows) + 1 tail (6 rows)
GROUPS = ((0, 4), (4, 4), (8, 4), (12, 3), (15, 1))  # softmax groups (start, size)
F32 = mybir.dt.float32
F16 = mybir.dt.float16


def _pack_weights(w: np.ndarray):
    """lhsT/lhsT_last [KD,128,128] (zero-padded K and M) + ob [128,128]."""

    def pack(nh):
        hh_n = nh + 2
        lhsT = np.zeros((KD, 128, 128), dtype=np.float32)
        for p in range(KD):
            for c in range(C):
                for kw in range(KW):
                    for hh in range(hh_n):
                        r = (c * KW + kw) * hh_n + hh
                        for hl in range(nh):
                            kh = hh - hl
                            if 0 <= kh < KH:
                                lhsT[p, r, hl * OC : hl * OC + OC] = w[:, c, p, kh, kw]
        return lhsT

    ob = np.zeros((128, 128), dtype=np.float32)
    for pp in range(128):
        g0 = (pp // OC) * OC
        ob[pp, g0 : g0 + OC] = 1.0
    return pack(8), pack(6), ob


def _pack_x5(x1: np.ndarray) -> np.ndarray:
    """x [3,24,128,128] f32 -> x5 [NBLK,128,24,126] f16 (padded rows zero)."""
    x5 = np.zeros((NBLK, 128, D, WO), dtype=np.float16)
    for b in range(NBLK):
        nh = 8 if b < NBLK - 1 else 6
        hh_n = nh + 2
        for c in range(C):
            for kw in range(KW):
                r0 = (c * KW + kw) * hh_n
                # [hh, d, w] <- x[c, d, 8b+hh, kw+w]
                x5[b, r0 : r0 + hh_n] = np.transpose(
                    x1[c, :, 8 * b : 8 * b + hh_n, kw : kw + WO], (1, 0, 2)
                )
    return x5


def build_program(reps: int = 1, stage2: str = "full"):
    """reps > 1 wraps the per-sample body in a hardware loop (dev timing only).
    stage2: none | exp | smmm | full (dev bisection of the softmax tail)."""
    nc = bacc.Bacc(
        "TRN2",
        target_bir_lowering=False,
        debug=False,
        enable_asserts=True,
        num_devices=NCORES,
    )
    x5_d = nc.dram_tensor("x5", [NBLK, 128, D, WO], F16, kind="ExternalInput").ap()
    lw_d = nc.dram_tensor("lw", [KD, 128, 128], F16, kind="ExternalInput").ap()
    lwl_d = nc.dram_tensor("lwl", [KD, 128, 128], F16, kind="ExternalInput").ap()
    ob_d = nc.dram_tensor("ob", [128, 128], F16, kind="ExternalInput").ap()
    y_d = nc.dram_tensor("y", [OC, HO, WO], F32, kind="ExternalOutput").ap()

    with tile.TileContext(nc) as tc:
        with (
            tc.tile_pool(name="const", bufs=1) as cpool,
            tc.tile_pool(name="xt", bufs=4) as xpool,
            tc.tile_pool(name="sm", bufs=3) as spool,
            tc.tile_pool(name="qps", bufs=6, space="PSUM") as qpool,
            tc.tile_pool(name="sps", bufs=2, space="PSUM") as smpool,
        ):
            lw_sb = cpool.tile([128, KD, 128], F16)
            nc.sync.dma_start(lw_sb[:], lw_d.rearrange("p r m -> r p m").bitcast(F16))
            lwl_sb = cpool.tile([128, KD, 128], F16)
            nc.sync.dma_start(lwl_sb[:], lwl_d.rearrange("p r m -> r p m").bitcast(F16))
            ob_sb = cpool.tile([128, 128], F16)
            nc.sync.dma_start(ob_sb[:], ob_d)

            def emit_body():
                state = {}  # per softmax group g: mn/et/st/ot tiles

                # Softmax over 4-block groups, software-pipelined across the
                # conv stream: each op is emitted a few blocks after its input
                # became available, so the in-order PE/DVE/ACT queues never
                # stall on cross-engine latency.
                def softmax_step(step, g):
                    g0, gsz = GROUPS[g]
                    if step == 0 and stage2 != "none":
                        et = spool.tile([128, gsz, WO], F16, tag="et", bufs=3, name=f"et{g}")
                        nc.scalar.activation(
                            et[:], state[g]["mn"][:], mybir.ActivationFunctionType.Exp
                        )
                        state[g]["et"] = et
                    if stage2 in ("none", "exp"):
                        return
                    eg = state[g]["et"][:]
                    if step == 1:
                        # group-sum broadcast to all 128 partitions in one MM:
                        # ob[k, p] = 1 iff k//16 == p//16
                        st = smpool.tile([128, gsz, WO], F32, tag="ss", name=f"st{g}")
                        nc.tensor.matmul(st[:], ob_sb[:], eg, start=True, stop=True)
                        state[g]["st"] = st
                    elif step == 2:
                        # softmax = exp(mn - ln(sum)); Ln/Exp ride the idle
                        # ScalarE instead of reciprocal+mult on the busy DVE
                        lt = spool.tile([128, gsz, WO], F32, tag="lt", bufs=2, name=f"lt{g}")
                        nc.scalar.activation(
                            lt[:], state[g]["st"][:], mybir.ActivationFunctionType.Ln
                        )
                        state[g]["lt"] = lt
                    elif step == 3:
                        dt = spool.tile([128, gsz, WO], F32, tag="dt", bufs=2, name=f"dt{g}")
                        nc.vector.tensor_tensor(
                            dt[:], state[g]["mn"][:], state[g]["lt"][:],
                            op=mybir.AluOpType.subtract,
                        )
                        state[g]["dt"] = dt
                    elif step == 4:
                        if stage2 == "smmm":
                            return
                        ot = spool.tile([128, gsz, WO], F32, tag="ot", bufs=2, name=f"ot{g}")
                        nc.scalar.activation(
                            ot[:], state[g]["dt"][:], mybir.ActivationFunctionType.Exp
                        )
                        for j in range(gsz):
                            bb_ = g0 + j
                            nh = 8 if bb_ < NBLK - 1 else 6
                            dst = y_d[:, 8 * bb_ : 8 * bb_ + nh, :].rearrange(
                                "oc h w -> h oc w"
                            )
                            nc.scalar.dma_start(dst, ot[: nh * OC, j, :])

                # schedule[B] = list of (step, g) to emit before conv block B
                # (exp right when its blocks' mins exist; the rest spaced a
                # few blocks later so the in-order engine queues never stall)
                schedule = {}
                for g, (g0, gsz) in enumerate(GROUPS):
                    end = g0 + gsz - 1
                    for step, off in enumerate((1, 4, 5, 6, 7)):
                        schedule.setdefault(end + off, []).append((step, g))

                blk2grp = {}
                for g, (g0, gsz) in enumerate(GROUPS):
                    for b in range(g0, g0 + gsz):
                        blk2grp[b] = g

                for b in range(NBLK):
                    m_n = 128
                    g_cur = blk2grp[b]
                    g0, gsz = GROUPS[g_cur]
                    if b == g0:
                        state[g_cur] = {
                            "mn": spool.tile([128, gsz, WO], F32, tag="mn", bufs=3, name=f"mn{g_cur}")
                        }
                    lw_t = lw_sb if b < NBLK - 1 else lwl_sb
                    for step, g in schedule.get(b, []):
                        softmax_step(step, g)

                    xt = xpool.tile([128, D, WO], F16, tag="xt")
                    nc.sync.dma_start(xt[:], x5_d[b].bitcast(F16))

                    mins = []
                    for q, (dq, nd) in enumerate(DQ):
                        pt = qpool.tile([m_n, nd, WO], F32, tag="q")
                        for p in range(KD):
                            nc.tensor.matmul(
                                pt[:],
                                lw_t[:, p, :m_n],
                                xt[:, dq + p : dq + p + nd, :],
                                start=(p == 0),
                                stop=(p == KD - 1),
                            )
                        qm = spool.tile([m_n, WO], F32, tag="qm", bufs=14)
                        nc.vector.tensor_reduce(
                            qm[:],
                            pt[:].rearrange("m j w -> m w j"),
                            axis=mybir.AxisListType.X,
                            op=mybir.AluOpType.min,
                        )
                        mins.append(qm)

                    t01 = spool.tile([m_n, WO], F32, tag="tm", bufs=10)
                    nc.vector.tensor_tensor(t01[:], mins[0][:], mins[1][:], op=mybir.AluOpType.min)
                    t23 = spool.tile([m_n, WO], F32, tag="tm", bufs=10)
                    nc.vector.tensor_tensor(t23[:], mins[2][:], mins[3][:], op=mybir.AluOpType.min)
                    t45 = spool.tile([m_n, WO], F32, tag="tm", bufs=10)
                    nc.vector.tensor_tensor(t45[:], mins[4][:], mins[5][:], op=mybir.AluOpType.min)
                    t03 = spool.tile([m_n, WO], F32, tag="tm", bufs=10)
                    nc.vector.tensor_tensor(t03[:], t01[:], t23[:], op=mybir.AluOpType.min)
                    nc.vector.tensor_tensor(
                        state[g_cur]["mn"][:m_n, b - g0, :], t03[:], t45[:],
                        op=mybir.AluOpType.min,
                    )

                # flush softmax steps scheduled past the last conv block
                # (block 15's weights are zero-padded to M=128, so its min
                # slice partitions 96..127 are exact zeros - finite for exp)
                for at in sorted(k for k in schedule if k >= NBLK):
                    for step, g in schedule[at]:
                        softmax_step(step, g)

            if reps == 1:
                emit_body()
            else:
                with tc.For_i(0, reps, 1, hint_engines=(mybir.EngineType.PE,)):
                    emit_body()

    nc.compile()
    return nc


@functools.lru_cache(maxsize=1)
def _program():
    return build_program()


def make_in_maps(x: np.ndarray, w: np.ndarray):
    lw, lwl, ob = _pack_weights(w)
    lw = lw.astype(np.float16)
    lwl = lwl.astype(np.float16)
    return [
        {"x5": _pack_x5(x[i]), "lw": lw, "lwl": lwl, "ob": ob.astype(np.float16)}
        for i in range(x.shape[0])
    ]


def kernel(x, conv_weight):
    x = np.ascontiguousarray(np.asarray(x, dtype=np.float32))
    w = np.ascontiguousarray(np.asarray(conv_weight, dtype=np.float32))
    assert x.shape == (NCORES, C, D, H, W), x.shape
    nc = _program()
    in_maps = make_in_maps(x, w)
    res = bass_utils.run_bass_kernel_spmd(nc, in_maps, core_ids=list(range(NCORES)))
    out = np.stack([res.results[i]["y"] for i in range(NCORES)])
    return out.astype(np.float32)


# revision 36
# speedup vs baseline: 1.2426x; 1.0351x over previous
"""Trainium2 Bass kernel for: Conv3d(3->16, k=3, VALID) -> min over depth -> softmax(channels).

Full inputs:  x [8, 3, 24, 128, 128] f32, conv_weight [16, 3, 3, 3, 3] f32
Full output:  [8, 16, 126, 126] f32
Sharding: data-parallel over batch, one sample per NeuronCore (8 cores).

Per-core scheme (H-packed im2col, fp16 matmuls, K padded to 128):
  - 16 h-blocks b; block handles h_out in [8b, 8b+8) (last block: 6 rows).
  - Host pre-packs x into x5 [16, 128, 24, 126] fp16:
    x5[b, (c*3+kw)*hh_n + hh, d, w] = x[c, d, 8b+hh, kw+w], rows 90.. zero.
    (3.75x replication; kd taps need no replication - depth lives in the free
    axis and is handled by shifted rhs offsets with PSUM accumulation.)
    One fully-contiguous 128-partition DMA per block => ~200 GB/s.
  - lhsT_p [128, M=(h_l,oc)] = W[oc, c, p, hh-h_l, kw] (0 <= hh-h_l < 3), p=kd.
    K=128 (zero-padded) keeps the PE xbus at full rate + enables FWL.
  - Depth quads DQ (5x4 + 1x2, no overlap waste); per quad one PSUM bank
    [M, nd, 126] accumulates the 3 kd passes (start/stop flags).
  - min over depth: per-quad DVE reduce_min + pairwise tree -> per-group mn.
  - softmax over the 16 channels per partition group of 16, as
    exp(mn - ln(sum exp)): exp/Ln on the otherwise-idle ScalarE, the
    channel-sum-and-broadcast as ONE fp16 PE matmul with a 0/1 block matrix
    (ob[k,p] = 1 iff k//16 == p//16), and only a subtract on VectorE.
    The whole chain is software-pipelined across the conv stream in groups
    of (4,4,4,3,1) h-blocks, each step emitted a few blocks after its input
    is ready so the in-order engine queues never stall; output DMAs ride
    the scalar queue so they never wait behind the big input loads.
"""

import functools
import os
import sys

import numpy as np

os.environ.setdefault("MYCRO_LOCAL_CACHE", "1")
if os.path.isdir("/opt/trn_rl_repo") and "/opt/trn_rl_repo" not in sys.path:
    sys.path.insert(0, "/opt/trn_rl_repo")

import concourse.bacc as bacc
import concourse.mybir as mybir
import concourse.tile as tile
from concourse import bass_utils

C, D, H, W = 3, 24, 128, 128
OC, KD, KH, KW = 16, 3, 3, 3
DO, HO, WO = D - 2, H - 2, W - 2  # 22, 126, 126
DQ = ((0, 4), (4, 4), (8, 4), (12, 4), (16, 4), (20, 2))  # (start, ndepth)
NCORES = 8
NBLK = 16  # h blocks: 15 full (8 rows) + 1 tail (6 rows)
GROUPS = ((0, 4), (4, 4), (8, 4), (12, 3), (15, 1))  # softmax groups (start, size)
F32 = mybir.dt.float32
F16 = mybir.dt.float16


def _pack_weights(w: np.ndarray):
    """lhsT/lhsT_last [KD,128,128] (zero-padded K and M) + ob [128,128]."""

    def pack(nh):
        hh_n = nh + 2
        lhsT = np.zeros((KD, 128, 128), dtype=np.float32)
        for p in range(KD):
            for c in range(C):
                for kw in range(KW):
                    for hh in range(hh_n):
                        r = (c * KW + kw) * hh_n + hh
                        for hl in range(nh):
                            kh = hh - hl
                            if 0 <= kh < KH:
                                lhsT[p, r, hl * OC : hl * OC + OC] = w[:, c, p, kh, kw]
        return lhsT

    ob = np.zeros((128, 128), dtype=np.float32)
    for pp in range(128):
        g0 = (pp // OC) * OC
        ob[pp, g0 : g0 + OC] = 1.0
    return pack(8), pack(6), ob


def _pack_x5(x1: np.ndarray) -> np.ndarray:
    """x [3,24,128,128] f32 -> x5 [NBLK,128,24,126] f16 (padded rows zero)."""
    x5 = np.zeros((NBLK, 128, D, WO), dtype=np.float16)
    for b in range(NBLK):
        nh = 8 if b < NBLK - 1 else 6
        hh_n = nh + 2
        for c in range(C):
            for kw in range(KW):
                r0 = (c * KW + kw) * hh_n
                # [hh, d, w] <- x[c, d, 8b+hh, kw+w]
                x5[b, r0 : r0 + hh_n] = np.transpose(
                    x1[c, :, 8 * b : 8 * b + hh_n, kw : kw + WO], (1, 0, 2)
                )
    return x5


def build_program(reps: int = 1, stage2: str = "full"):
    """reps > 1 wraps the per-sample body in a hardware loop (dev timing only).
    stage2: none | exp | smmm | full (dev bisection of the softmax tail)."""
    nc = bacc.Bacc(
        "TRN2",
        target_bir_lowering=False,
        debug=False,
        enable_asserts=True,
        num_devices=NCORES,
    )
    x5_d = nc.dram_tensor("x5", [NBLK, 128, D, WO], F16, kind="ExternalInput").ap()
    lw_d = nc.dram_tensor("lw", [KD, 128, 128], F16, kind="ExternalInput").ap()
    lwl_d = nc.dram_tensor("lwl", [KD, 128, 128], F16, kind="ExternalInput").ap()
    ob_d = nc.dram_tensor("ob", [128, 128], F16, kind="ExternalInput").ap()
    y_d = nc.dram_tensor("y", [OC, HO, WO], F32, kind="ExternalOutput").ap()

    with tile.TileContext(nc) as tc:
        with (
            tc.tile_pool(name="const", bufs=1) as cpool,
            tc.tile_pool(name="xt", bufs=4) as xpool,
            tc.tile_pool(name="sm", bufs=3) as spool,
            tc.tile_pool(name="qps", bufs=6, space="PSUM") as qpool,
            tc.tile_pool(name="sps", bufs=2, space="PSUM") as smpool,
        ):
            lw_sb = cpool.tile([128, KD, 128], F16)
            nc.sync.dma_start(lw_sb[:], lw_d.rearrange("p r m -> r p m").bitcast(F16))
            lwl_sb = cpool.tile([128, KD, 128], F16)
            nc.sync.dma_start(lwl_sb[:], lwl_d.rearrange("p r m -> r p m").bitcast(F16))
            ob_sb = cpool.tile([128, 128], F16)
            nc.sync.dma_start(ob_sb[:], ob_d)

            def emit_body():
                state = {}  # per softmax group g: mn/et/st/ot tiles

                # Softmax over 4-block groups, software-pipelined across the
                # conv stream: each op is emitted a few blocks after its input
                # became available, so the in-order PE/DVE/ACT queues never
                # stall on cross-engine latency.
                def softmax_step(step, g):
                    g0, gsz = GROUPS[g]
                    if step == 0 and stage2 != "none":
                        et = spool.tile([128, gsz, WO], F16, tag="et", bufs=3, name=f"et{g}")
                        nc.scalar.activation(
                            et[:], state[g]["mn"][:], mybir.ActivationFunctionType.Exp
                        )
                        state[g]["et"] = et
                    if stage2 in ("none", "exp"):
                        return
                    eg = state[g]["et"][:]
                    if step == 1:
                        # group-sum broadcast to all 128 partitions in one MM:
                        # ob[k, p] = 1 iff k//16 == p//16
                        st = smpool.tile([128, gsz, WO], F32, tag="ss", name=f"st{g}")
                        nc.tensor.matmul(st[:], ob_sb[:], eg, start=True, stop=True)
                        state[g]["st"] = st
                    elif step == 2:
                        # softmax = exp(mn - ln(sum)); Ln/Exp ride the idle
                        # ScalarE instead of reciprocal+mult on the busy DVE
                        lt = spool.tile([128, gsz, WO], F32, tag="lt", bufs=2, name=f"lt{g}")
                        nc.scalar.activation(
                            lt[:], state[g]["st"][:], mybir.ActivationFunctionType.Ln
                        )
                        state[g]["lt"] = lt
                    elif step == 3:
                        dt = spool.tile([128, gsz, WO], F32, tag="dt", bufs=2, name=f"dt{g}")
                        nc.vector.tensor_tensor(
                            dt[:], state[g]["mn"][:], state[g]["lt"][:],
                            op=mybir.AluOpType.subtract,
                        )
                        state[g]["dt"] = dt
                    elif step == 4:
                        if stage2 == "smmm":
                            return
                        ot = spool.tile([128, gsz, WO], F32, tag="ot", bufs=2, name=f"ot{g}")
                        nc.scalar.activation(
                            ot[:], state[g]["dt"][:], mybir.ActivationFunctionType.Exp
                        )
                        for j in range(gsz):
                            bb_ = g0 + j
                            nh = 8 if bb_ < NBLK - 1 else 6
                            dst = y_d[:, 8 * bb_ : 8 * bb_ + nh, :].rearrange(
                                "oc h w -> h oc w"
                            )
                            nc.scalar.dma_start(dst, ot[: nh * OC, j, :])

                # schedule[B] = list of (step, g) to emit before conv block B
                # (exp right when its blocks' mins exist; the rest spaced a
                # few blocks later so the in-order engine queues never stall)
                schedule = {}
                for g, (g0, gsz) in enumerate(GROUPS):
                    end = g0 + gsz - 1
                    for step, off in enumerate((1, 4, 5, 6, 7)):
                        schedule.setdefault(end + off, []).append((step, g))

                blk2grp = {}
                for g, (g0, gsz) in enumerate(GROUPS):
                    for b in range(g0, g0 + gsz):
                        blk2grp[b] = g

                for b in range(NBLK):
                    m_n = 128
                    g_cur = blk2grp[b]
                    g0, gsz = GROUPS[g_cur]
                    if b == g0:
                        state[g_cur] = {
                            "mn": spool.tile([128, gsz, WO], F32, tag="mn", bufs=3, name=f"mn{g_cur}")
                        }
                    lw_t = lw_sb if b < NBLK - 1 else lwl_sb
                    for step, g in schedule.get(b, []):
                        softmax_step(step, g)

                    xt = xpool.tile([128, D, WO], F16, tag="xt")
                    nc.sync.dma_start(xt[:], x5_d[b].bitcast(F16))

                    mins = []
                    for q, (dq, nd) in enumerate(DQ):
                        pt = qpool.tile([m_n, nd, WO], F32, tag="q")
                        for p in range(KD):
                            nc.tensor.matmul(
                                pt[:],
                                lw_t[:, p, :m_n],
                                xt[:, dq + p : dq + p + nd, :],
                                start=(p == 0),
                                stop=(p == KD - 1),
                            )
                        qm = spool.tile([m_n, WO], F32, tag="qm", bufs=14)
                        nc.vector.tensor_reduce(
                            qm[:],
                            pt[:].rearrange("m j w -> m w j"),
                            axis=mybir.AxisListType.X,
                            op=mybir.AluOpType.min,
                        )
                        mins.append(qm)

                    t01 = spool.tile([m_n, WO], F32, tag="tm", bufs=10)
                    nc.vector.tensor_tensor(t01[:], mins[0][:], mins[1][:], op=mybir.AluOpType.min)
                    t23 = spool.tile([m_n, WO], F32, tag="tm", bufs=10)
                    nc.vector.tensor_tensor(t23[:], mins[2][:], mins[3][:], op=mybir.AluOpType.min)
                    t45 = spool.tile([m_n, WO], F32, tag="tm", bufs=10)
                    nc.vector.tensor_tensor(t45[:], mins[4][:], mins[5][:], op=mybir.AluOpType.min)
                    t03 = spool.tile([m_n, WO], F32, tag="tm", bufs=10)
                    nc.vector.tensor_tensor(t03[:], t01[:], t23[:], op=mybir.AluOpType.min)
                    nc.vector.tensor_tensor(
                        state[g_cur]["mn"][:m_n, b - g0, :], t03[:], t45[:],
                        op=mybir.AluOpType.min,
                    )

                # flush softmax steps scheduled past the last conv block
                # (block 15's weights are zero-padded to M=128, so its min
                # slice partitions 96..127 are exact zeros - finite for exp)
                for at in sorted(k for k in schedule if k >= NBLK):
                    for step, g in schedule[at]:
                        softmax_step(step, g)

            if reps == 1:
                emit_body()
            else:
                with tc.For_i(0, reps, 1, hint_engines=(mybir.EngineType.PE,), staggered_reset=True):
                    emit_body()

    nc.compile()
    return nc


@functools.lru_cache(maxsize=1)
def _program():
    return build_program()


def make_in_maps(x: np.ndarray, w: np.ndarray):
    lw, lwl, ob = _pack_weights(w)
    lw = lw.astype(np.float16)
    lwl = lwl.astype(np.float16)
    return [
        {"x5": _pack_x5(x[i]), "lw": lw, "lwl": lwl, "ob": ob.astype(np.float16)}
        for i in range(x.shape[0])
    ]


def kernel(x, conv_weight):
    x = np.ascontiguousarray(np.asarray(x, dtype=np.float32))
    w = np.ascontiguousarray(np.asarray(conv_weight, dtype=np.float32))
    assert x.shape == (NCORES, C, D, H, W), x.shape
    nc = _program()
    in_maps = make_in_maps(x, w)
    res = bass_utils.run_bass_kernel_spmd(nc, in_maps, core_ids=list(range(NCORES)))
    out = np.stack([res.results[i]["y"] for i in range(NCORES)])
    return out.astype(np.float32)
